# revision 24
# baseline (speedup 1.0000x reference)
"""Trainium2 Bass kernel for multi-head attention (dense transformer block).

Reference computation (per batch element):
    qkv = x @ w_qkv                      # [N, 3C]
    q, k, v = split heads (H=12, HD=64); q *= HD**-0.5
    out = softmax(q k^T) v               # full [N, N] scores
    out = merge_heads(out) @ w_proj + b_proj

Distribution: pure data parallel over the batch dim — B=8 batch elements,
8 NeuronCores, one element per core.  Weights are replicated.  No
collectives are needed; each core computes its full [2048, 768] output.

Per-core compute strategy (all matmuls bf16, fp32 PSUM accumulation):
  * x is cast f32->bf16 by a SWDGE DMA into a DRAM scratch, then DMA-xbar
    transposed into SBUF as xT [768, 2048] (feature-on-partition layout).
  * qkT = w_qk^T @ x^T -> [1536, 2048]: q/k for a head PAIR live in one
    128-partition tile (head A on partitions 0-63, head B on 64-127), so
    the K=64 score matmuls auto-pack as 64x128 row tiles of the PE array.
  * v = x @ w_v -> [2048, 768] natural layout (keys on partitions), which
    is exactly the lhsT needed for the attnV matmuls.
  * scoresT[m, n] = kT^T qT per head: keys on partitions, queries on the
    free dim.  exp() runs on ScalarE straight out of PSUM at FD=1024 (a
    head pair's [128, 2x512] chunk per instruction), with the 1/8
    softmax scale folded into the activation's free affine.  No max
    subtraction: scaled scores are ~N(0,1) so exp never overflows.
  * attnV: outT_h = v_h^T @ A_T^h accumulated over the 16 key tiles.  The
    two heads of a pair auto-pack as 128x64 column tiles (head A ->
    PSUM partitions 0-63, head B -> 64-127) sharing one PSUM bank.
  * softmax denominators: ones^T @ A_T matmuls, four heads (a "quad")
    packed as 128x32 column tiles into one PSUM bank.
  * normalization (divide by denominators) is applied at the attnV
    PSUM->SBUF eviction: reciprocal on DVE, broadcast across partitions
    via a DMA bounce, one tensor_tensor multiply.
  * final = outT^T @ w_proj with b_proj preloaded into PSUM by a K=1
    ones-matmul, evicted f32 and DMA'd out.
"""

import os

import numpy as np

import concourse.bass as bass
import concourse.mybir as mybir
from concourse import bacc, bass_utils
from concourse.tile import TileContext

F32 = mybir.dt.float32
BF16 = mybir.dt.bfloat16
AF = mybir.ActivationFunctionType

B, N, C = 8, 2048, 768
H, HD = 12, 64
SCALE = HD ** -0.5  # folded into the exp activation
P = 128
NT = N // P          # 16 token tiles
CT = C // P          # 6 feature tiles
NCHUNK = 4           # query chunks of 512
QW = N // NCHUNK     # 512


def build_nc() -> bass.Bass:
    nc = bacc.Bacc(None)
    x = nc.declare_dram_parameter("x", [N, C], F32, isOutput=False)
    w_qkv = nc.declare_dram_parameter("w_qkv", [C, 3 * C], F32, isOutput=False)
    w_proj = nc.declare_dram_parameter("w_proj", [C, C], F32, isOutput=False)
    b_proj = nc.declare_dram_parameter("b_proj", [C], F32, isOutput=False)
    out = nc.declare_dram_parameter("out", [N, C], F32, isOutput=True)

    with TileContext(nc) as tc:
        with (
            tc.tile_pool(name="const", bufs=1) as cpool,
            tc.tile_pool(name="dram", bufs=1, space="DRAM") as dpool,
            tc.tile_pool(name="rdram", bufs=2, space="DRAM") as rdpool,
            tc.tile_pool(name="at", bufs=6) as at_pool,
            tc.tile_pool(name="recip", bufs=2) as recip_pool,
            tc.tile_pool(name="rbc", bufs=2) as rbc_pool,
            tc.tile_pool(name="fin", bufs=2) as fin_pool,
            tc.tile_pool(name="psc", bufs=2, space="PSUM") as psum_sc,
            tc.tile_pool(name="pav", bufs=2, space="PSUM") as psum_av,
            tc.tile_pool(name="psum1", bufs=1, space="PSUM") as psum_sums,
            tc.tile_pool(name="pproj", bufs=1, space="PSUM") as psum_proj,
        ):
            # ---- persistent SBUF tensors -------------------------------
            w_qkv_sb = cpool.tile([P, CT, 3 * C], BF16, tag="wqkv")
            wproj_sb = cpool.tile([P, CT, C], BF16, tag="wproj")
            b_bc = cpool.tile([P, C], F32, tag="bias")  # bias bcast to 128 rows
            ones128 = cpool.tile([P, 1], BF16, tag="ones128")
            xT = cpool.tile([P, CT, N], BF16, tag="xT")
            qkT = cpool.tile([P, 12, N], BF16, tag="qkT")  # 12 = q(6 pairs)+k(6)
            v4 = cpool.tile([P, NT, C], BF16, tag="v4")
            outT = cpool.tile([P, CT, N], BF16, tag="outT")

            # ---- phase 0: load + cast + transpose ----------------------
            # interleave the x-cast chain with per-chunk w_qkv casts on the
            # SWDGE queue so the first qkT matmul's inputs (xT ct0 + w ct0)
            # are both ready within a few us; w_proj/bias load last.
            nc.any.memset(ones128[:], 1.0)
            nc.gpsimd.dma_start(
                out=w_qkv_sb[:], in_=w_qkv.rearrange("(o p) j -> p o j", p=P)
            )
            nc.gpsimd.dma_start(
                out=wproj_sb[:], in_=w_proj.rearrange("(o p) j -> p o j", p=P)
            )
            nc.sync.dma_start(
                out=b_bc[:], in_=b_proj[None, :].to_broadcast((P, C))
            )
            x_bf = dpool.tile([N, C], BF16)
            for ct in range(CT):
                csl = slice(ct * P, (ct + 1) * P)
                # per-column-chunk cast so each transpose starts early
                nc.gpsimd.dma_start(out=x_bf[:, csl], in_=x[:, csl])
                nc.sync.dma_start_transpose(xT[:, ct, :], x_bf[:, csl])

            # ---- phase 1: qkv projections ------------------------------
            # qkT[j, n] for j in [0, 1536): q rows 0-767, k rows 768-1535
            def emit_qk_group(jt: int, c4: int):
                ps = psum_sc.tile([P, 1024], F32, tag="sc")
                for ct in range(CT):
                    nc.tensor.matmul(
                        ps[:, 0:QW],
                        lhsT=w_qkv_sb[:, ct, jt * P : (jt + 1) * P],
                        rhs=xT[:, ct, c4 * QW : (c4 + 1) * QW],
                        start=(ct == 0),
                        stop=(ct == CT - 1),
                    )
                nc.vector.tensor_copy(
                    out=qkT[:, jt, c4 * QW : (c4 + 1) * QW], in_=ps[:, 0:QW]
                )

            # v natural layout: v[n, e] = sum_c x[n, c] w_qkv[c, 1536 + e]
            def emit_v_group(nt: int, eo: int, ew: int):
                ps = psum_sc.tile([P, 1024], F32, tag="sc")
                for ct in range(CT):
                    nc.tensor.matmul(
                        ps[:, 0:ew],
                        lhsT=xT[:, ct, nt * P : (nt + 1) * P],
                        rhs=w_qkv_sb[:, ct, 2 * C + eo : 2 * C + eo + ew],
                        start=(ct == 0),
                        stop=(ct == CT - 1),
                    )
                nc.vector.tensor_copy(out=v4[:, nt, eo : eo + ew], in_=ps[:, 0:ew])

            # upfront: all of kT (every score matmul needs all key tiles) and
            # qT for chunk 0.  v tiles and later chunks' qT are emitted
            # just-in-time inside the attention loops to shorten the serial
            # PE-only prologue.
            for jt in range(6, 12):
                for c4 in range(NCHUNK):
                    emit_qk_group(jt, c4)
            for jt in range(6):
                emit_qk_group(jt, 0)
            # chunk c's qT groups are emitted during chunk c-1, quad 2
            qt_slots = {2: 0, 5: 1, 8: 2, 11: 3, 13: 4, 15: 5}  # m -> jt

            # ---- phase 2+3: attention + projection ---------------------
            def emit_proj_group(nt: int, eo: int, ew: int):
                """final[nt-tile, eo:eo+ew] = outT^T w_proj + b."""
                ps = psum_proj.tile([P, 512], F32, tag="proj")
                for ct in range(CT):
                    nc.tensor.matmul(
                        ps[:, 0:ew],
                        lhsT=outT[:, ct, nt * P : (nt + 1) * P],
                        rhs=wproj_sb[:, ct, eo : eo + ew],
                        start=(ct == 0),
                        stop=(ct == CT - 1),
                    )
                fs = fin_pool.tile([P, 512], F32, tag="fin")
                nc.vector.tensor_tensor(
                    fs[:, 0:ew], ps[:, 0:ew], b_bc[:, eo : eo + ew],
                    mybir.AluOpType.add,
                )
                nc.sync.dma_start(
                    out=out[nt * P : (nt + 1) * P, eo : eo + ew], in_=fs[:, 0:ew]
                )

            # proj work for chunk c-1 is spread through chunk c's m-loops
            # (slots on quad 0/1 at fixed m) to avoid starving ScalarE.
            proj_slots = {  # (quad, m) -> slot index 0..7
                (0, 3): 0, (0, 7): 1, (0, 11): 2, (0, 14): 3,
                (1, 3): 4, (1, 7): 5, (1, 11): 6, (1, 14): 7,
            }

            def emit_proj_slot(c_done: int, slot: int):
                nt = c_done * 4 + slot // 2
                eo, ew = ((0, 512), (512, 256))[slot % 2]
                emit_proj_group(nt, eo, ew)

            for c in range(NCHUNK):
                qsl = slice(c * QW, (c + 1) * QW)
                for quad in range(3):
                    attn_ps = [
                        psum_av.tile([P, QW], F32, tag="av", name=f"av{pp}")
                        for pp in range(2)
                    ]
                    sums_ps = psum_sums.tile([P, QW], F32, tag="sums")
                    # only rows {0,32,64,96} get matmul results; init the rest
                    # so the full-tile reciprocal below reads defined memory
                    nc.vector.memset(sums_ps[:], 1.0)
                    for m in range(NT):
                        msl = slice(m * P, (m + 1) * P)
                        # just-in-time v tiles (chunk 0) and next-chunk qT
                        if c == 0 and quad == 0:
                            emit_v_group(m, 0, 512)
                        elif c == 0 and quad == 1:
                            emit_v_group(m, 512, 256)
                        elif quad == 2 and c < NCHUNK - 1 and m in qt_slots:
                            emit_qk_group(qt_slots[m], c + 1)
                        at_pair = []
                        for pp in range(2):
                            pair = 2 * quad + pp
                            sc = psum_sc.tile([P, 1024], F32, tag="sc")
                            # scoresT chunk: keys msl on partitions, queries
                            # qsl on free dim.  Head A rows 0-63, head B
                            # rows 64-127 -> auto row-tiled 64x128 pair.
                            nc.tensor.matmul(
                                sc[:, 0:QW],
                                lhsT=qkT[0:64, 6 + pair, msl],
                                rhs=qkT[0:64, pair, qsl],
                                start=True,
                                stop=True,
                            )
                            nc.tensor.matmul(
                                sc[:, QW : 2 * QW],
                                lhsT=qkT[64:128, 6 + pair, msl],
                                rhs=qkT[64:128, pair, qsl],
                                start=True,
                                stop=True,
                            )
                            at = at_pool.tile([P, 1024], BF16, tag="at")
                            nc.scalar.activation(at[:], sc[:], AF.Exp, scale=SCALE)
                            at_pair.append(at)
                        for pp in range(2):
                            pair = 2 * quad + pp
                            at = at_pair[pp]
                            for hh in range(2):
                                h = 2 * pair + hh
                                # attnV: col-tiled head pair, one PSUM bank
                                nc.tensor.matmul(
                                    attn_ps[pp][hh * 64 : (hh + 1) * 64, :],
                                    lhsT=v4[:, m, h * 64 : (h + 1) * 64],
                                    rhs=at[:, hh * QW : (hh + 1) * QW],
                                    start=(m == 0),
                                    stop=(m == NT - 1),
                                    # the sim's group-check view is partition-
                                    # blind; only the first col tile of the
                                    # shared bank may do the bookkeeping
                                    skip_group_check=(hh != 0),
                                )
                        for pp in range(2):
                            at = at_pair[pp]
                            for hh in range(2):
                                k4 = 2 * pp + hh
                                # denominators: 4 heads as 128x32 col tiles
                                nc.tensor.matmul(
                                    sums_ps[k4 * 32 : k4 * 32 + 1, :],
                                    lhsT=ones128[:, 0:1],
                                    rhs=at[:, hh * QW : (hh + 1) * QW],
                                    start=(m == 0),
                                    stop=(m == NT - 1),
                                    skip_group_check=(k4 != 0),
                                    tile_position=(0, k4 * 32),
                                )
                        if c > 0 and (quad, m) in proj_slots:
                            emit_proj_slot(c - 1, proj_slots[(quad, m)])

                    # ---- normalize + evict this (quad, chunk) ----------
                    recip_sb = recip_pool.tile([P, QW], F32, tag="recip")
                    nc.vector.reciprocal(recip_sb[:], sums_ps[:])
                    # bounce the 4 live rows through DRAM so a DMA can
                    # broadcast them across partitions
                    r_dram = rdpool.tile([4, QW], F32)
                    nc.sync.dma_start(out=r_dram[:], in_=recip_sb[0:97:32, :])
                    for pp in range(2):
                        rbc = rbc_pool.tile([P, QW], F32, tag="rbc")
                        nc.sync.dma_start(
                            out=rbc[0:64, :],
                            in_=r_dram[2 * pp : 2 * pp + 1, :].to_broadcast((64, QW)),
                        )
                        nc.sync.dma_start(
                            out=rbc[64:128, :],
                            in_=r_dram[2 * pp + 1 : 2 * pp + 2, :].to_broadcast(
                                (64, QW)
                            ),
                        )
                        nc.vector.tensor_tensor(
                            outT[:, 2 * quad + pp, qsl],
                            attn_ps[pp][:],
                            rbc[:],
                            mybir.AluOpType.mult,
                        )
            # tail: proj for the last chunk
            for slot in range(8):
                emit_proj_slot(NCHUNK - 1, slot)

    nc.compile()
    return nc


_NC_CACHE: list = []


def _get_nc() -> bass.Bass:
    if not _NC_CACHE:
        _NC_CACHE.append(build_nc())
    return _NC_CACHE[0]


def run(inputs: dict, trace: bool = False):
    """Run on 8 NeuronCores.  Returns (out [B,N,C] f32, exec_time_ns|None)."""
    nc = _get_nc()
    x = np.ascontiguousarray(np.asarray(inputs["x"], dtype=np.float32))
    w_qkv = np.ascontiguousarray(np.asarray(inputs["w_qkv"], dtype=np.float32))
    w_proj = np.ascontiguousarray(np.asarray(inputs["w_proj"], dtype=np.float32))
    b_proj = np.ascontiguousarray(np.asarray(inputs["b_proj"], dtype=np.float32))
    in_maps = [
        {"x": x[i], "w_qkv": w_qkv, "w_proj": w_proj, "b_proj": b_proj}
        for i in range(B)
    ]
    try:
        res = bass_utils.run_bass_kernel_spmd(
            nc, in_maps, core_ids=list(range(B)), trace=trace
        )
    except ModuleNotFoundError:
        # NTFF profile hook unavailable in this image; run without trace
        res = bass_utils.run_bass_kernel_spmd(
            nc, in_maps, core_ids=list(range(B)), trace=False
        )
    out = np.stack([res.results[i]["out"] for i in range(B)], axis=0)
    return out.astype(np.float32), res.exec_time_ns


def kernel(x, w_qkv, w_proj, b_proj):
    trace = os.environ.get("BASS_KERNEL_TRACE", "0") == "1"
    out, _ = run(
        {"x": x, "w_qkv": w_qkv, "w_proj": w_proj, "b_proj": b_proj}, trace=trace
    )
    return out
